# revision 45
# baseline (speedup 1.0000x reference)
"""MixerDiffAttention Trainium2 kernel (v4 — fp8 DoubleRow projection).

Sharding: 8 cores = 8 head-pairs (tensor parallel over head-pair dim).
Each core processes BOTH batches for its head-pair: the per-core weight
slice (768 qkv cols + 256 gate cols) stays SBUF-resident, and each core
produces the disjoint output slice y[:, :, hp*256:(hp+1)*256].

Key scheduling facts (from the timeline cost model): matmul cost =
out_free_size x dtype_rate (contraction depth is free; fp8e4 DoubleRow
runs at 0.5 cycles/row AND contracts 2x128 rows per instruction),
engines execute in per-engine program order, any PE idle resets the
p-state ramp (next 3us at 2x cycle time), and reopened tile pools
carry WAR deps on the previous scope's readers -- so the P2 SBUF pools
are hoisted to the outer scope. q/k transposes run on the PE (identity
matmul, 53ns) deferred one tile behind the rope chain; all-DMA-xbar
transposes saturate SP.SEQ's ~0.4us-per-issue in-order path and stall
P2 by ~10us, but the LAST tile's go via DMA so the PE stream flows
straight into P2's scores.

Projection precision: x and 64*W are split host-side into fp8(e4m3)
hi+lo pairs (hi = fp8(a), lo = fp8(a - hi)). z = x_hi@W_hi +
(x_hi@W_lo + x_lo@W_hi), the correction pair riding the two DoubleRow
slots of one instruction per 128-feature block; the dropped lo@lo term
is ~0.13%. Measured on the real inputs this is MORE accurate than the
bf16 path (proj rms 1.2e-3 vs 2.4e-3) at 0.75x the PE cost. The
uniform 64x output scale is absorbed by the q/k RMSNorm, the final
group RMSNorm (for v and the gate product), and an exp scale of -1/64
in the SiLU sigmoid.

Per core, per batch:
  Phase 1 (per 128-token tile; x and W stream in as packed fp8 hi|lo,
    256-token 512B-run DMAs; tile 0's corrections deferred behind tile
    1's hi-only mains so the warmup isn't gated on the W lo-planes):
    qk projection matmuls first, then v|gate (the qk-stats chain starts
    half a tile early); qk staged to SBUF f32 (frees the PSUM bank);
    RMSNorm stats via ACT Square+accum; rstd via DVE Quake-seed Newton
    (2 it); RoPE on DVE in f32; single bf16 rounding at the rstd-scale;
    feature-major q/k via deferred PE transposes + ACT copyback; v
    (+ones column for softmax row sums) and raw gate copied by ACT one
    tile late; SiLU gate via sigma=1/(1+exp(-g)): ACT Exp, Pool add,
    DVE recip-approx, Pool mult (all off the critical chain).
  Phase 2 (flat software-pipelined group stream): score matmuls for
    group g+1 are emitted BEFORE the AV matmuls of group g, so the PE
    never sits behind ACT's exp. Exps are batched 2 score-pairs per ACT
    instruction (exp_and_friends table set only -> no table swaps); the
    causal-diagonal slice is masked multiplicatively after exp (exact
    0/1 f32 on DVE); the diagonal AV block that is fully causal-masked
    is skipped outright. The epilogue overlaps attention: y1's normalize
    starts when var0's accumulators finish; the diff combine, SiLU
    gating, and group RMSNorm (rsqrt absorbs the 1-lambda_init factor)
    finish after var1, with sum-of-squares on DVE (ACT on the tail
    chunk where ACT is idle).
"""
import sys
sys.path.insert(0, "/opt/trn_rl_repo")
import numpy as np
import concourse.bass as bass
from concourse import bacc
import concourse.tile as tile
from concourse import mybir
from concourse.bass_utils import run_bass_kernel_spmd

F32 = mybir.dt.float32
F32R = mybir.dt.float32r
BF16 = mybir.dt.bfloat16
F8 = mybir.dt.float8e4
DR = mybir.MatmulPerfMode.DoubleRow
AF = mybir.ActivationFunctionType
ALU = mybir.AluOpType
WSCALE = 64.0

B, T, D, HD = 2, 2048, 2048, 128
KT = D // 128          # 16 contraction tiles
TT = T // 128          # 16 token tiles
CH = 256               # query-chunk width in phase 2
NCH = T // CH          # 8 chunks
N_CORES = 8
LAMBDA_INIT = 0.8 - 0.6 * float(np.exp(-0.3 * 6))
ONE_MINUS_LI = 1.0 - LAMBDA_INIT
SCALE = float(HD ** -0.5)
EPS = 1e-6


def _bcast_mid(ap, n):
    # [P, F] AP -> [P, n, F] with a zero-stride middle dim
    return bass.AP(tensor=ap.tensor, offset=ap.offset,
                   ap=[ap.ap[0], [0, n], *ap.ap[1:]])


def _rsqrt_dve(nc, pool, ss_ap, width, mean_div, tag, iters=2, eps=EPS):
    """rstd = (ss/mean_div + EPS) ** -0.5 entirely on DVE.

    Quake-III bit-trick seed + Newton iterations (2 it: ~5e-6 rel err;
    1 it: ~1.7e-3 max rel err); avoids ACT Ln/Sqrt so the whole kernel
    stays inside one ACT table set."""
    I32 = mybir.dt.int32
    ms = pool.tile([128, width], F32, name=tag + "_ms")
    nc.vector.tensor_scalar(out=ms[:], in0=ss_ap, scalar1=1.0 / mean_div,
                            scalar2=eps, op0=ALU.mult, op1=ALU.add)
    iv = pool.tile([128, width], I32, name=tag + "_iv")
    nc.vector.tensor_scalar(out=iv[:], in0=ms[:].bitcast(I32), scalar1=1,
                            scalar2=None, op0=ALU.logical_shift_right)
    y = pool.tile([128, width], F32, name=tag + "_y")
    nc.vector.tensor_scalar(out=y[:].bitcast(I32), in0=iv[:], scalar1=-1,
                            scalar2=0x5F3759DF, op0=ALU.mult, op1=ALU.add)
    a = pool.tile([128, width], F32, name=tag + "_a")
    u = pool.tile([128, width], F32, name=tag + "_u")
    for _ in range(iters):
        nc.vector.tensor_mul(a[:], y[:], y[:])
        nc.vector.tensor_mul(a[:], a[:], ms[:])
        nc.vector.tensor_scalar(out=u[:], in0=a[:], scalar1=-0.5, scalar2=1.5,
                                op0=ALU.mult, op1=ALU.add)
        nc.vector.tensor_mul(y[:], y[:], u[:])
    return y


def build(tt=TT, nb=B, phases=2):
    nch = tt * 128 // CH
    nc = bacc.Bacc("TRN2", target_bir_lowering=False, debug=False,
                   num_devices=N_CORES)
    # x / W in fp8 hi+lo pairs: x packed [D, S, 2(hi,lo), 256] so one
    # 512B-run DMA per feature row pulls both planes of a super tile;
    # W packed [D, 2(lo,hi), 1024] so the correction matmul's moving AP
    # [lo|hi] pairs against the stationary x [hi|lo] DoubleRow slots.
    xt_d = nc.dram_tensor("xt", [nb, D, tt * 128 // 256, 2, 256], F8,
                          kind="ExternalInput").ap()
    w_d = nc.dram_tensor("wcat", [D, 2, 1024], F8, kind="ExternalInput").ap()
    cos_d = nc.dram_tensor("cos", [tt * 128, 64], F32, kind="ExternalInput").ap()
    sin_d = nc.dram_tensor("sin", [tt * 128, 64], F32, kind="ExternalInput").ap()
    mask_d = nc.dram_tensor("masks", [128, 2, CH], F32R, kind="ExternalInput").ap()
    id_d = nc.dram_tensor("ident", [128, 128], BF16, kind="ExternalInput").ap()
    ones_d = nc.dram_tensor("ones", [128, 4], F32R, kind="ExternalInput").ap()
    y_d = nc.dram_tensor("y", [nb, tt * 128, 256], F32, kind="ExternalOutput").ap()

    with tile.TileContext(nc) as tc:
        with tc.tile_pool(name="bigs", bufs=1) as bigs, \
             tc.tile_pool(name="consts", bufs=1) as consts, \
             tc.tile_pool(name="p2s", bufs=5) as p2s, \
             tc.tile_pool(name="p2e", bufs=5) as p2e, \
             tc.tile_pool(name="xtp", bufs=5) as xtp:
            # ---- weights first: the k=0..1 slices gate the first matmul ----
            wcat = bigs.tile([128, KT, 2, 1024], F8)
            w_v = w_d.rearrange("(k p) i c -> p k i c", p=128)

            # ---- 256-token "super tile" loads (fp8 hi+lo, 512B runs) ----
            xT_pre = {}

            def load_super(b, s):
                xv = xt_d[b].rearrange("(k p) s i t -> p k s i t", p=128)
                xp = xtp.tile([128, KT, 2, 256], F8, name="xT_s")
                for kh in range(2):
                    nc.sync.dma_start(
                        xp[:, kh * 8:(kh + 1) * 8, :, :],
                        xv[:, kh * 8:(kh + 1) * 8, s, :, :])
                xT_pre[(b, s)] = xp

            # first x blocks + W hi-planes gate the first (main-term)
            # matmuls; W lo-planes are only needed by the corrections,
            # which are deferred for the first two tiles (see p1 loop) --
            # so hi loads lead and the PE starts within ~2us
            xv0 = xt_d[0].rearrange("(k p) s i t -> p k s i t", p=128)
            xp0 = xtp.tile([128, KT, 2, 256], F8, name="xT_s")
            nc.sync.dma_start(xp0[:, 0:2, :, :], xv0[:, 0:2, 0, :, :])
            nc.sync.dma_start(wcat[:, 0:2, 1, :], w_v[:, 0:2, 1, :])
            nc.sync.dma_start(xp0[:, 2:8, :, :], xv0[:, 2:8, 0, :, :])
            nc.sync.dma_start(wcat[:, 2:8, 1, :], w_v[:, 2:8, 1, :])
            nc.sync.dma_start(xp0[:, 8:16, :, :], xv0[:, 8:16, 0, :, :])
            nc.sync.dma_start(wcat[:, 8:16, 1, :], w_v[:, 8:16, 1, :])
            xT_pre[(0, 0)] = xp0
            nc.sync.dma_start(wcat[:, 0:8, 0, :], w_v[:, 0:8, 0, :])
            nc.sync.dma_start(wcat[:, 8:16, 0, :], w_v[:, 8:16, 0, :])
            load_super(0, 1)
            # ---- small constants (needed only after the first projection) ----
            cos_sb = consts.tile([128, tt, 64], F32)
            nc.sync.dma_start(cos_sb[:], cos_d.rearrange("(t p) f -> p t f", p=128))
            sin_sb = consts.tile([128, tt, 64], F32)
            nc.sync.dma_start(sin_sb[:], sin_d.rearrange("(t p) f -> p t f", p=128))
            id_sb = consts.tile([128, 128], BF16)
            nc.sync.dma_start(id_sb[:], id_d)
            load_super(0, 2)
            mask_sb = consts.tile([128, 2, CH], F32R)
            nc.sync.dma_start(mask_sb[:], mask_d)
            ones_sb = consts.tile([128, 4], F32R)
            nc.sync.dma_start(ones_sb[:], ones_d)
            load_super(0, 3)

            # ---- per-batch persistent (reused sequentially) ----
            qkT = bigs.tile([128, tt, 4, 128], BF16)    # t-major; rows q1,q2,k1,k2
            v_sb = bigs.tile([128, tt, 260], F32R)      # [tok, v(256)|1|0 pad]
            g_sb = bigs.tile([128, tt, 256], F32)       # gate (raw -> silu'd JIT)

            for b in range(nb):
                # ================= Phase 1 =================
                with tc.tile_pool(name="p1t", bufs=3) as p1t, \
                     tc.tile_pool(name="mm_ps", bufs=3, space="PSUM") as mm_ps, \
                     tc.tile_pool(name="tp_ps", bufs=2, space="PSUM") as tp_ps:
                    # ones column for every tile in one strided write
                    nc.vector.tensor_copy(v_sb[:, :, 256:260],
                                          _bcast_mid(ones_sb[:], tt))

                    # q/k transposes on the PE (identity matmul, 53ns each)
                    # with an ACT copyback: all-DMA transposes saturate
                    # SP.SEQ's in-order ~0.4us-per-issue path and drain so
                    # late that P2(b0) stalls ~10us. Deferred one tile so
                    # the PE never waits on the rope chain.
                    def p1_transp(t, qrot):
                        tp = tp_ps.tile([128, 4, 128], BF16, name="tp")
                        for h in range(4):
                            nc.tensor.matmul(tp[:, h, :], qrot[:, h, :],
                                             id_sb[:], is_transpose=True)
                        nc.scalar.copy(qkT[:, t, :, :], tp[:])

                    pending_vg = None
                    pending_tp = None

                    def p1_vg(t, vg_ps):
                        # v / raw gate copies + SiLU gate; deferred one tile
                        # so the next tile's squares lead the ACT queue
                        nc.scalar.copy(v_sb[:, t, 0:256], vg_ps[:, 0:256])
                        nc.scalar.copy(g_sb[:, t, :], vg_ps[:, 256:512])
                        ge = p1t.tile([128, 256], F32, name="ge")
                        # g_sb holds 64*z; sigmoid wants exp(-z)
                        nc.scalar.activation(ge[:], g_sb[:, t, :], AF.Exp,
                                             scale=-1.0 / WSCALE)
                        gd = p1t.tile([128, 256], F32, name="gd")
                        nc.gpsimd.tensor_scalar(out=gd[:], in0=ge[:], scalar1=1.0,
                                                scalar2=None, op0=ALU.add)
                        gr = p1t.tile([128, 256], F32, name="gr")
                        nc.vector.reciprocal_approx_fast(out=gr[:], in_=gd[:])
                        nc.gpsimd.tensor_mul(g_sb[:, t, :], g_sb[:, t, :], gr[:])

                    def p1_main(xT_t, xsl, ps, cols):
                        # main term: x_hi @ W_hi, 2 k-blocks per inst
                        for r in range(KT // 2):
                            nc.tensor.matmul(
                                ps[:], xT_t[:, 2 * r:2 * r + 2, 0, xsl],
                                wcat[:, 2 * r:2 * r + 2, 1, cols],
                                start=(r == 0), stop=False, perf_mode=DR)

                    def p1_corr(xT_t, xsl, ps, cols):
                        # correction: x_hi@W_lo + x_lo@W_hi via the two
                        # DoubleRow slots of one inst per k-block
                        for r in range(KT):
                            nc.tensor.matmul(
                                ps[:], xT_t[:, r, :, xsl], wcat[:, r, :, cols],
                                start=False, stop=(r == KT - 1), perf_mode=DR)

                    def p1_post(t, qk_ps, vg_ps, last=False):
                        nonlocal pending_tp, pending_vg
                        # ---- q/k rmsnorm stats FIRST on ACT (they gate the
                        # rsqrt -> qrot -> transpose chain). For the LAST
                        # tile they move to DVE: its rope chain feeds only
                        # the chunk-7 DMA transpose (~40us of slack), and
                        # clearing ACT lets P2's first exps start ~1us
                        # earlier ----
                        ss = p1t.tile([128, 4], F32, name="ss")
                        if last:
                            sqv = p1t.tile([128, 4, 128], F32, name="sqv",
                                           bufs=1)
                            qkv = qk_ps[:].rearrange("p (h d) -> p h d", h=4)
                            nc.vector.tensor_mul(sqv[:], qkv, qkv)
                            nc.vector.tensor_reduce(
                                ss[:], sqv[:], axis=mybir.AxisListType.X,
                                op=ALU.add)
                        else:
                            sq_scr = p1t.tile([128, 128], F32, name="sq_scr")
                            for h in range(4):
                                nc.scalar.activation(
                                    sq_scr[:], qk_ps[:, h * 128:(h + 1) * 128],
                                    AF.Square, accum_out=ss[:, h:h + 1])
                        # ---- stage qk to SBUF in f32 (frees the PSUM bank
                        # early; single bf16 rounding happens at qrot) ----
                        qksb = p1t.tile([128, 4, 128], F32, name="qksb")
                        nc.scalar.copy(qksb[:],
                                       qk_ps[:].rearrange("p (h d) -> p h d", h=4))
                        h1, h2 = qksb[:, :, 0:64], qksb[:, :, 64:128]
                        cos_b = _bcast_mid(cos_sb[:, t, :], 4)
                        sin_b = _bcast_mid(sin_sb[:, t, :], 4)
                        ra = p1t.tile([128, 4, 64], F32, name="ra")
                        rb = p1t.tile([128, 4, 64], F32, name="rb")
                        rot = p1t.tile([128, 4, 128], F32, name="rot")
                        nc.vector.tensor_mul(ra[:], h1, cos_b)
                        nc.vector.tensor_mul(rb[:], h2, sin_b)
                        nc.vector.tensor_add(rot[:, :, 0:64], ra[:], rb[:])
                        nc.vector.tensor_mul(ra[:], h2, cos_b)
                        nc.vector.tensor_mul(rb[:], h1, sin_b)
                        nc.vector.tensor_sub(rot[:, :, 64:128], ra[:], rb[:])
                        rstd = _rsqrt_dve(nc, p1t, ss[:], 4, HD, "rq", iters=2)
                        qrot = p1t.tile([128, 4, 128], BF16, name="qrot")
                        for h in range(4):
                            nc.vector.tensor_scalar_mul(qrot[:, h, :], in0=rot[:, h, :],
                                                        scalar1=rstd[:, h:h + 1])
                        if pending_tp is not None:
                            p1_transp(*pending_tp)
                        pending_tp = (t, qrot)
                        if vg_ps is not None:
                            if pending_vg is not None:
                                p1_vg(*pending_vg)
                            pending_vg = (t, vg_ps)

                    pend_corr = []
                    for t in range(tt):
                        s, half = t // 2, t % 2
                        if half == 0 and (b, s) not in xT_pre:
                            load_super(b, s)
                        xT_t = xT_pre[(b, s)]
                        if half == 1:
                            del xT_pre[(b, s)]
                            # prefetch 3 supers ahead
                            if s + 3 < tt // 2 and (b, s + 3) not in xT_pre:
                                load_super(b, s + 3)
                        xsl = slice(half * 128, half * 128 + 128)
                        qk_ps = mm_ps.tile([128, 512], F32, name="qk_ps")
                        vg_ps = mm_ps.tile([128, 512], F32, name="vg_ps")
                        if b == 0 and t < 1:
                            # warmup: corrections need the W lo-planes, which
                            # are still streaming in -- run tiles 0-2's mains
                            # (hi-only) first so the PE isn't DMA-gated
                            p1_main(xT_t, xsl, qk_ps, slice(0, 512))
                            p1_main(xT_t, xsl, vg_ps, slice(512, 1024))
                            pend_corr.append((t, xT_t, xsl, qk_ps, vg_ps))
                            continue
                        if pend_corr:
                            p1_main(xT_t, xsl, qk_ps, slice(0, 512))
                            p1_main(xT_t, xsl, vg_ps, slice(512, 1024))
                            for pt, pxT, pxsl, pqk, pvg in pend_corr:
                                p1_corr(pxT, pxsl, pqk, slice(0, 512))
                                p1_corr(pxT, pxsl, pvg, slice(512, 1024))
                                p1_post(pt, pqk, pvg)
                            pend_corr = []
                            p1_corr(xT_t, xsl, qk_ps, slice(0, 512))
                            p1_corr(xT_t, xsl, vg_ps, slice(512, 1024))
                            p1_post(t, qk_ps, vg_ps)
                            continue
                        p1_main(xT_t, xsl, qk_ps, slice(0, 512))
                        p1_corr(xT_t, xsl, qk_ps, slice(0, 512))
                        p1_main(xT_t, xsl, vg_ps, slice(512, 1024))
                        p1_corr(xT_t, xsl, vg_ps, slice(512, 1024))
                        p1_post(t, qk_ps, vg_ps)
                    # vg first: its ACT copies are ready immediately and
                    # must not queue behind P2's first exps. The LAST tile's
                    # transposes go on the DMA xbar: a PE transpose would
                    # sit in the in-order PE stream waiting ~2.4us for the
                    # rope chain, stalling P2's first scores; its qkT slice
                    # is only read by chunk 7, ~40us later.
                    p1_vg(*pending_vg)
                    pending_vg = None
                    lt, lqrot = pending_tp
                    pending_tp = None
                    for h in range(4):
                        nc.sync.dma_start_transpose(qkT[:, lt, h, :],
                                                    lqrot[:, h, :])
                # prefetch next batch's first supers during phase 2
                if b + 1 < nb:
                    for s in range(2):
                        load_super(b + 1, s)
                if phases < 2:
                    with tc.tile_pool(name="dump", bufs=2) as dump:
                        for t in range(tt):
                            d_t = dump.tile([128, 256], F32, name="d_t")
                            nc.vector.tensor_copy(d_t[:], v_sb[:, t, 0:256])
                            nc.vector.tensor_add(d_t[:], d_t[:], g_sb[:, t, :])
                            nc.sync.dma_start(
                                y_d[b, t * 128:(t + 1) * 128, :], d_t[:])
                    continue
                # ================= Phase 2 =================
                with tc.tile_pool(name="sc_ps", bufs=2, space="PSUM") as sc_ps, \
                     tc.tile_pool(name="av_ps", bufs=4, space="PSUM") as av_ps:
                    # --- job list: groups of <=2 score pairs; diagonal is its
                    # own group (needs the causal mask) -------------------
                    groups = []
                    for c in range(nch):
                        per_var = []
                        for var in range(2):
                            gs = []
                            prs_all = list(range(c + 1))
                            for i in range(0, len(prs_all), 2):
                                grp = prs_all[i:i + 2]
                                gs.append((c, var, grp, c in grp))
                            per_var.append(gs)
                        # interleave var streams; keep var0's diag before
                        # var1's diag so the pre-epilogue still leads
                        n = len(per_var[0])
                        for i in range(n):
                            groups.append(per_var[0][i])
                            groups.append(per_var[1][i])

                    sc_tiles = {}

                    def emit_sc(gi):
                        c, var, prs, diag = groups[gi]
                        scp = sc_ps.tile([128, 4, CH], F32, name="sc")
                        qch = qkT[:, 2 * c:2 * c + 2, var, :]
                        for pi, jp in enumerate(prs):
                            for jj in range(2):
                                nc.tensor.matmul(
                                    scp[:, 2 * pi + jj, :],
                                    qkT[:, 2 * jp + jj, 2 + var, :],
                                    qch, start=True, stop=True)
                        sc_tiles[gi] = scp

                    emit_sc(0)
                    yps = {}
                    for gi, (c, var, prs, diag) in enumerate(groups):
                        if var == 0 and prs[0] == 0:
                            for v2 in range(2):
                                for m in range(2):
                                    yps[(v2, m)] = av_ps.tile([128, 258], F32,
                                                              name="yacc")
                        n = 2 * len(prs)
                        scp = sc_tiles.pop(gi)
                        probs = p2s.tile([128, 4, CH], F32R, name="probs")
                        nc.scalar.activation(probs[:, 0:n, :], scp[:, 0:n, :],
                                             AF.Exp, scale=SCALE)
                        if diag:
                            pi = prs.index(c)
                            nc.vector.tensor_mul(
                                probs[:, 2 * pi:2 * pi + 2, :],
                                probs[:, 2 * pi:2 * pi + 2, :], mask_sb[:])
                        # emit next group's scores ahead of this group's AV
                        if gi + 1 < len(groups):
                            emit_sc(gi + 1)
                        for pi, jp in enumerate(prs):
                            for jj in range(2):
                                j = 2 * jp + jj
                                for m in range(2):
                                    if j == 2 * c + 1 and m == 0:
                                        # fully-masked diagonal block: probs
                                        # are exactly zero there -> skip
                                        continue
                                    nc.tensor.matmul(
                                        yps[(var, m)][:],
                                        probs[:, 2 * pi + jj, m * 128:(m + 1) * 128],
                                        v_sb[:, j, 0:258],
                                        start=(j == 0),
                                        stop=(j == 2 * c + 1 - (1 - m)))
                        if diag and var == 0:
                            # var0 accumulators are complete: start the
                            # normalize of y1 while var1's attention runs
                            pre_ep = {}
                            for m in range(2):
                                y1p = yps[(0, m)]
                                r1 = p2e.tile([128, 1], F32, name="r1")
                                nc.vector.reciprocal(r1[:], y1p[:, 256:257])
                                t1 = p2e.tile([128, 256], F32, name="t1")
                                nc.vector.tensor_scalar_mul(
                                    t1[:], in0=y1p[:, 0:256], scalar1=r1[:])
                                pre_ep[m] = t1
                        if not (diag and var == 1):
                            continue
                        # ---- epilogue for chunk c ----
                        ssy = p2e.tile([128, 2], F32, name="ssy")
                        ygs = []
                        for m in range(2):
                            y2p = yps[(1, m)]
                            # v col 257 = -1/lam -> r2n is one recip away
                            r2n = p2e.tile([128, 1], F32, name="r2n")
                            nc.vector.reciprocal(r2n[:], y2p[:, 257:258])
                            t1 = pre_ep[m]
                            yt = p2e.tile([128, 256], F32, name="yt")
                            nc.vector.scalar_tensor_tensor(
                                yt[:], y2p[:, 0:256], r2n[:], t1[:],
                                op0=ALU.mult, op1=ALU.add)
                            yg = p2e.tile([128, 256], F32, name="yg", bufs=2)
                            nc.vector.tensor_mul(yg[:], yt[:],
                                                 g_sb[:, 2 * c + m, :])
                            if c == nch - 1 and m == 0:
                                # tail chunk: m=0 stats on the otherwise-idle
                                # ACT, m=1 on DVE -- the two run in parallel
                                sq = p2e.tile([128, 256], F32, name="sq2")
                                nc.scalar.activation(sq[:], yg[:], AF.Square,
                                                     accum_out=ssy[:, m:m + 1])
                            else:
                                sq = p2e.tile([128, 256], F32, name="sq2")
                                nc.vector.tensor_mul(sq[:], yg[:], yg[:])
                                nc.vector.tensor_reduce(
                                    ssy[:, m:m + 1], sq[:],
                                    axis=mybir.AxisListType.X, op=ALU.add)
                            ygs.append(yg)
                        # rsy absorbs the (1-lambda_init) factor:
                        # (ms/C^2)^-0.5 = C * ms^-0.5. The tail chunk drops
                        # to 1 Newton iter (<=1.7e-3 rel on 256 tokens) to
                        # shorten the end-of-kernel drain chain.
                        CI2 = 1.0 / (ONE_MINUS_LI * ONE_MINUS_LI)
                        rsy = _rsqrt_dve(nc, p2e, ssy[:], 2, 256 / CI2, "ry",
                                         iters=1 if c == nch - 1 else 2,
                                         eps=EPS * CI2)
                        out_t = p2e.tile([128, 2, 256], F32, name="out_t",
                                         bufs=2)
                        for m in range(2):
                            nc.vector.tensor_scalar_mul(
                                out_t[:, m, :], in0=ygs[m][:],
                                scalar1=rsy[:, m:m + 1])
                            if c == nch - 1:
                                # tail: per-half DMA so the first issue
                                # overlaps the second half's scale
                                nc.sync.dma_start(
                                    y_d[b, (2 * c + m) * 128:
                                        (2 * c + m + 1) * 128, :],
                                    out_t[:, m, :])
                        if c != nch - 1:
                            nc.sync.dma_start(
                                y_d[b, 2 * c * 128:(2 * c + 2) * 128, :]
                                .rearrange("(m p) c -> p m c", p=128),
                                out_t[:])
    nc.compile()
    return nc


_NC = None


def prep_in_maps(hidden_states, W_qkv, lambda_q1, lambda_k1, lambda_q2,
                 lambda_k2, W_g):
    import ml_dtypes
    bf16 = ml_dtypes.bfloat16
    f8 = ml_dtypes.float8_e4m3
    x = np.asarray(hidden_states, dtype=np.float32)
    xt = np.ascontiguousarray(x.transpose(0, 2, 1))        # [B, D, T] f32
    x_hi = xt.astype(f8)
    x_lo = (xt - x_hi.astype(np.float32)).astype(f8)
    # pack [B, D, S, 2, 256]: hi and lo planes adjacent per 256-tok super
    xt_p = np.empty((B, D, T // 256, 2, 256), dtype=f8)
    xt_p[:, :, :, 0, :] = x_hi.reshape(B, D, T // 256, 256)
    xt_p[:, :, :, 1, :] = x_lo.reshape(B, D, T // 256, 256)
    W_qkv = np.asarray(W_qkv, dtype=np.float32)
    W_g = np.asarray(W_g, dtype=np.float32)

    t_ar = np.arange(T, dtype=np.float32)
    inv_freq = (1.0 / 10000.0 ** (np.arange(0, HD, 2, dtype=np.float32) / HD)
                ).astype(np.float32)
    freqs = np.outer(t_ar, inv_freq).astype(np.float32)
    cos = np.cos(freqs).astype(np.float32)
    sin = np.sin(freqs).astype(np.float32)

    # multiplicative 0/1 causal mask (applied to probs AFTER exp)
    masks = np.empty((128, 2, CH), dtype=np.float32)
    kk = np.arange(128)[:, None]
    qq = np.arange(CH)[None, :]
    for m in range(2):
        masks[:, m, :] = np.where(m * 128 + kk <= qq, 1.0, 0.0)
    
    ident = np.eye(128, dtype=bf16)

    lam1 = np.exp(np.sum(np.asarray(lambda_q1, np.float32)
                         * np.asarray(lambda_k1, np.float32), axis=-1))
    lam2 = np.exp(np.sum(np.asarray(lambda_q2, np.float32)
                         * np.asarray(lambda_k2, np.float32), axis=-1))
    lam = (lam1 - lam2 + LAMBDA_INIT).astype(np.float32)   # [8]

    in_maps = []
    for c in range(N_CORES):
        base = 2 * c * 384
        w_cols = [
            W_qkv[:, base:base + 128],            # q1
            W_qkv[:, base + 384:base + 512],      # q2
            W_qkv[:, base + 128:base + 256],      # k1
            W_qkv[:, base + 512:base + 640],      # k2
            W_qkv[:, base + 256:base + 384],      # v1
            W_qkv[:, base + 640:base + 768],      # v2
            W_g[:, c * 256:(c + 1) * 256],        # gate
        ]
        wc = np.concatenate(w_cols, axis=1) * WSCALE        # [D, 1024] f32
        w_hi = wc.astype(f8)
        w_lo = (wc - w_hi.astype(np.float32)).astype(f8)
        # pack [D, 2, 1024]: slot0 = LO, slot1 = HI (correction AP order)
        wcat = np.empty((D, 2, 1024), dtype=f8)
        wcat[:, 0, :] = w_lo
        wcat[:, 1, :] = w_hi
        ones = np.zeros((128, 4), dtype=np.float32)
        ones[:, 0] = 1.0
        ones[:, 1] = -1.0 / lam[c]
        in_maps.append({
            "xt": xt_p[:, :, :, :, :], "wcat": wcat, "cos": cos, "sin": sin,
            "masks": masks, "ident": ident, "ones": ones,
        })

    return in_maps


def kernel(hidden_states, W_qkv, lambda_q1, lambda_k1, lambda_q2, lambda_k2,
           W_g, **run_kwargs):
    global _NC
    if _NC is None:
        _NC = build()
    in_maps = prep_in_maps(hidden_states, W_qkv, lambda_q1, lambda_k1,
                           lambda_q2, lambda_k2, W_g)
    res = run_bass_kernel_spmd(_NC, in_maps, core_ids=list(range(N_CORES)),
                               **run_kwargs)
    out = np.empty((B, T, D), dtype=np.float32)
    for c in range(N_CORES):
        out[:, :, c * 256:(c + 1) * 256] = res.results[c]["y"]
    if run_kwargs:
        return out, res
    return out



# revision 56
# speedup vs baseline: 1.0033x; 1.0033x over previous
"""MixerDiffAttention Trainium2 kernel (v4 — fp8 DoubleRow projection).

Sharding: 8 cores = 8 head-pairs (tensor parallel over head-pair dim).
Each core processes BOTH batches for its head-pair: the per-core weight
slice (768 qkv cols + 256 gate cols) stays SBUF-resident, and each core
produces the disjoint output slice y[:, :, hp*256:(hp+1)*256].

Key scheduling facts (from the timeline cost model): matmul cost =
out_free_size x dtype_rate (contraction depth is free; fp8e4 DoubleRow
runs at 0.5 cycles/row AND contracts 2x128 rows per instruction),
engines execute in per-engine program order, any PE idle resets the
p-state ramp (next 3us at 2x cycle time), and reopened tile pools
carry WAR deps on the previous scope's readers -- so the P2 SBUF pools
are hoisted to the outer scope. q/k transposes run on the PE (identity
matmul, 53ns) deferred one tile behind the rope chain; all-DMA-xbar
transposes saturate SP.SEQ's ~0.4us-per-issue in-order path and stall
P2 by ~10us, but the LAST tile's go via DMA so the PE stream flows
straight into P2's scores.

Projection precision: x and 64*W are split host-side into fp8(e4m3)
hi+lo pairs (hi = fp8(a), lo = fp8(a - hi)). z = x_hi@W_hi +
(x_hi@W_lo + x_lo@W_hi), the correction pair riding the two DoubleRow
slots of one instruction per 128-feature block; the dropped lo@lo term
is ~0.13%. Measured on the real inputs this is MORE accurate than the
bf16 path (proj rms 1.2e-3 vs 2.4e-3) at 0.75x the PE cost. The
uniform 64x output scale is absorbed by the q/k RMSNorm, the final
group RMSNorm (for v and the gate product), and an exp scale of -1/64
in the SiLU sigmoid.

Per core, per batch:
  Phase 1 (per 128-token tile; x and W stream in as packed fp8 hi|lo,
    256-token 512B-run DMAs; tile 0's corrections deferred behind tile
    1's hi-only mains so the warmup isn't gated on the W lo-planes):
    qk projection matmuls first, then v|gate (the qk-stats chain starts
    half a tile early); qk staged to SBUF f32 (frees the PSUM bank);
    RMSNorm stats via ACT Square+accum; rstd via DVE Quake-seed Newton
    (2 it); RoPE on DVE in f32; single bf16 rounding at the rstd-scale;
    feature-major q/k via deferred PE transposes + ACT copyback; v
    (+ones column for softmax row sums) and raw gate copied by ACT one
    tile late; SiLU gate via sigma=1/(1+exp(-g)): ACT Exp, Pool add,
    DVE recip-approx, Pool mult (all off the critical chain).
  Phase 2 (flat software-pipelined group stream): score matmuls for
    group g+1 are emitted BEFORE the AV matmuls of group g, so the PE
    never sits behind ACT's exp. Exps are batched 2 score-pairs per ACT
    instruction (exp_and_friends table set only -> no table swaps); the
    causal-diagonal slice is masked multiplicatively after exp (exact
    0/1 f32 on DVE); the diagonal AV block that is fully causal-masked
    is skipped outright. The epilogue overlaps attention: y1's normalize
    starts when var0's accumulators finish; the diff combine, SiLU
    gating, and group RMSNorm (rsqrt absorbs the 1-lambda_init factor)
    finish after var1, with sum-of-squares on DVE (ACT on the tail
    chunk where ACT is idle).
"""
import sys
sys.path.insert(0, "/opt/trn_rl_repo")
import numpy as np
import concourse.bass as bass
from concourse import bacc
import concourse.tile as tile
from concourse import mybir
from concourse.bass_utils import run_bass_kernel_spmd

F32 = mybir.dt.float32
F32R = mybir.dt.float32r
BF16 = mybir.dt.bfloat16
F8 = mybir.dt.float8e4
DR = mybir.MatmulPerfMode.DoubleRow
AF = mybir.ActivationFunctionType
ALU = mybir.AluOpType
WSCALE = 64.0

B, T, D, HD = 2, 2048, 2048, 128
KT = D // 128          # 16 contraction tiles
TT = T // 128          # 16 token tiles
CH = 256               # query-chunk width in phase 2
NCH = T // CH          # 8 chunks
N_CORES = 8
LAMBDA_INIT = 0.8 - 0.6 * float(np.exp(-0.3 * 6))
ONE_MINUS_LI = 1.0 - LAMBDA_INIT
SCALE = float(HD ** -0.5)
EPS = 1e-6


def _bcast_mid(ap, n):
    # [P, F] AP -> [P, n, F] with a zero-stride middle dim
    return bass.AP(tensor=ap.tensor, offset=ap.offset,
                   ap=[ap.ap[0], [0, n], *ap.ap[1:]])


def _rsqrt_dve(nc, pool, ss_ap, width, mean_div, tag, iters=2, eps=EPS):
    """rstd = (ss/mean_div + EPS) ** -0.5 entirely on DVE.

    Quake-III bit-trick seed + Newton iterations (2 it: ~5e-6 rel err;
    1 it: ~1.7e-3 max rel err); avoids ACT Ln/Sqrt so the whole kernel
    stays inside one ACT table set."""
    I32 = mybir.dt.int32
    ms = pool.tile([128, width], F32, name=tag + "_ms")
    nc.vector.tensor_scalar(out=ms[:], in0=ss_ap, scalar1=1.0 / mean_div,
                            scalar2=eps, op0=ALU.mult, op1=ALU.add)
    iv = pool.tile([128, width], I32, name=tag + "_iv")
    nc.vector.tensor_scalar(out=iv[:], in0=ms[:].bitcast(I32), scalar1=1,
                            scalar2=None, op0=ALU.logical_shift_right)
    y = pool.tile([128, width], F32, name=tag + "_y")
    nc.vector.tensor_scalar(out=y[:].bitcast(I32), in0=iv[:], scalar1=-1,
                            scalar2=0x5F3759DF, op0=ALU.mult, op1=ALU.add)
    a = pool.tile([128, width], F32, name=tag + "_a")
    u = pool.tile([128, width], F32, name=tag + "_u")
    for _ in range(iters):
        nc.vector.tensor_mul(a[:], y[:], y[:])
        nc.vector.tensor_mul(a[:], a[:], ms[:])
        nc.vector.tensor_scalar(out=u[:], in0=a[:], scalar1=-0.5, scalar2=1.5,
                                op0=ALU.mult, op1=ALU.add)
        nc.vector.tensor_mul(y[:], y[:], u[:])
    return y


def build(tt=TT, nb=B, phases=2):
    nch = tt * 128 // CH
    nc = bacc.Bacc("TRN2", target_bir_lowering=False, debug=False,
                   num_devices=N_CORES)
    # x / W in fp8 hi+lo pairs: x packed [D, S, 2(hi,lo), 256] so one
    # 512B-run DMA per feature row pulls both planes of a super tile;
    # W packed [D, 2(lo,hi), 1024] so the correction matmul's moving AP
    # [lo|hi] pairs against the stationary x [hi|lo] DoubleRow slots.
    xt_d = nc.dram_tensor("xt", [nb, D, tt * 128 // 256, 2, 256], F8,
                          kind="ExternalInput").ap()
    w_d = nc.dram_tensor("wcat", [D, 2, 1024], F8, kind="ExternalInput").ap()
    cos_d = nc.dram_tensor("cos", [tt * 128, 64], F32, kind="ExternalInput").ap()
    sin_d = nc.dram_tensor("sin", [tt * 128, 64], F32, kind="ExternalInput").ap()
    mask_d = nc.dram_tensor("masks", [128, 2, CH], F32R, kind="ExternalInput").ap()
    id_d = nc.dram_tensor("ident", [128, 128], BF16, kind="ExternalInput").ap()
    ones_d = nc.dram_tensor("ones", [128, 4], F32R, kind="ExternalInput").ap()
    y_d = nc.dram_tensor("y", [nb, tt * 128, 256], F32, kind="ExternalOutput").ap()

    with tile.TileContext(nc) as tc:
        with tc.tile_pool(name="bigs", bufs=1) as bigs, \
             tc.tile_pool(name="consts", bufs=1) as consts, \
             tc.tile_pool(name="p2s", bufs=5) as p2s, \
             tc.tile_pool(name="p2e", bufs=5) as p2e, \
             tc.tile_pool(name="xtp", bufs=5) as xtp:
            # ---- weights first: the k=0..1 slices gate the first matmul ----
            wcat = bigs.tile([128, KT, 2, 1024], F8)
            w_v = w_d.rearrange("(k p) i c -> p k i c", p=128)

            # ---- 256-token "super tile" loads (fp8 hi+lo, 512B runs) ----
            xT_pre = {}

            def load_super(b, s):
                xv = xt_d[b].rearrange("(k p) s i t -> p k s i t", p=128)
                xp = xtp.tile([128, KT, 2, 256], F8, name="xT_s")
                for kh in range(2):
                    nc.sync.dma_start(
                        xp[:, kh * 8:(kh + 1) * 8, :, :],
                        xv[:, kh * 8:(kh + 1) * 8, s, :, :])
                xT_pre[(b, s)] = xp

            # first x blocks + W hi-planes gate the first (main-term)
            # matmuls; W lo-planes are only needed by the corrections,
            # which are deferred for the first two tiles (see p1 loop) --
            # so hi loads lead and the PE starts within ~2us
            xv0 = xt_d[0].rearrange("(k p) s i t -> p k s i t", p=128)
            xp0 = xtp.tile([128, KT, 2, 256], F8, name="xT_s")
            nc.sync.dma_start(xp0[:, 0:2, :, :], xv0[:, 0:2, 0, :, :])
            nc.sync.dma_start(wcat[:, 0:2, 1, :], w_v[:, 0:2, 1, :])
            nc.sync.dma_start(xp0[:, 2:8, :, :], xv0[:, 2:8, 0, :, :])
            nc.sync.dma_start(wcat[:, 2:8, 1, :], w_v[:, 2:8, 1, :])
            nc.sync.dma_start(xp0[:, 8:16, :, :], xv0[:, 8:16, 0, :, :])
            nc.sync.dma_start(wcat[:, 8:16, 1, :], w_v[:, 8:16, 1, :])
            xT_pre[(0, 0)] = xp0
            nc.sync.dma_start(wcat[:, 0:8, 0, :], w_v[:, 0:8, 0, :])
            nc.sync.dma_start(wcat[:, 8:16, 0, :], w_v[:, 8:16, 0, :])
            load_super(0, 1)
            # ---- small constants (needed only after the first projection) ----
            cos_sb = consts.tile([128, tt, 64], F32)
            nc.sync.dma_start(cos_sb[:], cos_d.rearrange("(t p) f -> p t f", p=128))
            sin_sb = consts.tile([128, tt, 64], F32)
            nc.sync.dma_start(sin_sb[:], sin_d.rearrange("(t p) f -> p t f", p=128))
            id_sb = consts.tile([128, 128], BF16)
            nc.sync.dma_start(id_sb[:], id_d)
            load_super(0, 2)
            mask_sb = consts.tile([128, 2, CH], F32R)
            nc.sync.dma_start(mask_sb[:], mask_d)
            ones_sb = consts.tile([128, 4], F32R)
            nc.sync.dma_start(ones_sb[:], ones_d)
            load_super(0, 3)

            # ---- per-batch persistent (reused sequentially) ----
            qkT = bigs.tile([128, tt, 4, 128], BF16)    # t-major; rows q1,q2,k1,k2
            v_sb = bigs.tile([128, tt, 260], F32R)      # [tok, v(256)|1|0 pad]
            g_sb = bigs.tile([128, tt, 256], F32)       # gate (raw -> silu'd JIT)

            for b in range(nb):
                # ================= Phase 1 =================
                with tc.tile_pool(name="p1t", bufs=3) as p1t, \
                     tc.tile_pool(name="mm_ps", bufs=3, space="PSUM") as mm_ps, \
                     tc.tile_pool(name="tp_ps", bufs=2, space="PSUM") as tp_ps:
                    # ones column for every tile in one strided write
                    nc.vector.tensor_copy(v_sb[:, :, 256:260],
                                          _bcast_mid(ones_sb[:], tt))

                    # q/k transposes on the PE (identity matmul, 53ns each)
                    # with an ACT copyback: all-DMA transposes saturate
                    # SP.SEQ's in-order ~0.4us-per-issue path and drain so
                    # late that P2(b0) stalls ~10us. Deferred one tile so
                    # the PE never waits on the rope chain.
                    def p1_transp(t, qrot):
                        tp = tp_ps.tile([128, 4, 128], BF16, name="tp")
                        for h in range(4):
                            nc.tensor.matmul(tp[:, h, :], qrot[:, h, :],
                                             id_sb[:], is_transpose=True)
                        nc.scalar.copy(qkT[:, t, :, :], tp[:])

                    pending_vg = None
                    pending_tp = None

                    def p1_vg(t, vg_ps, pool=None, defer_silu=False):
                        # v / raw gate copies + SiLU gate; deferred one tile
                        # so the next tile's squares lead the ACT queue
                        pool = pool or p1t
                        nc.scalar.copy(v_sb[:, t, 0:256], vg_ps[:, 0:256])
                        nc.scalar.copy(g_sb[:, t, :], vg_ps[:, 256:512])

                        def silu(bufs=3):
                            ge = pool.tile([128, 256], F32, name="ge",
                                           bufs=bufs)
                            # g_sb holds 64*z; sigmoid wants exp(-z)
                            nc.scalar.activation(ge[:], g_sb[:, t, :], AF.Exp,
                                                 scale=-1.0 / WSCALE)
                            gd = pool.tile([128, 256], F32, name="gd",
                                           bufs=bufs)
                            nc.gpsimd.tensor_scalar(out=gd[:], in0=ge[:],
                                                    scalar1=1.0, scalar2=None,
                                                    op0=ALU.add)
                            gr = pool.tile([128, 256], F32, name="gr",
                                           bufs=bufs)
                            nc.vector.reciprocal_approx_fast(out=gr[:],
                                                             in_=gd[:])
                            nc.gpsimd.tensor_mul(g_sb[:, t, :], g_sb[:, t, :],
                                                 gr[:])

                        if defer_silu:
                            return silu
                        silu()

                    def p1_main(xT_t, xsl, ps, cols):
                        # main term: x_hi @ W_hi, 2 k-blocks per inst
                        for r in range(KT // 2):
                            nc.tensor.matmul(
                                ps[:], xT_t[:, 2 * r:2 * r + 2, 0, xsl],
                                wcat[:, 2 * r:2 * r + 2, 1, cols],
                                start=(r == 0), stop=False, perf_mode=DR)

                    def p1_corr(xT_t, xsl, ps, cols):
                        # correction: x_hi@W_lo + x_lo@W_hi via the two
                        # DoubleRow slots of one inst per k-block
                        for r in range(KT):
                            nc.tensor.matmul(
                                ps[:], xT_t[:, r, :, xsl], wcat[:, r, :, cols],
                                start=False, stop=(r == KT - 1), perf_mode=DR)

                    def p1_post(t, qk_ps, vg_ps, last=False):
                        nonlocal pending_tp, pending_vg
                        # ---- q/k rmsnorm stats FIRST on ACT (they gate the
                        # rsqrt -> qrot -> transpose chain). For the LAST
                        # tile they move to DVE: its rope chain feeds only
                        # the chunk-7 DMA transpose (~40us of slack), and
                        # clearing ACT lets P2's first exps start ~1us
                        # earlier ----
                        ss = p1t.tile([128, 4], F32, name="ss")
                        if last:
                            sqv = p1t.tile([128, 4, 128], F32, name="sqv",
                                           bufs=1)
                            qkv = qk_ps[:].rearrange("p (h d) -> p h d", h=4)
                            nc.vector.tensor_mul(sqv[:], qkv, qkv)
                            nc.vector.tensor_reduce(
                                ss[:], sqv[:], axis=mybir.AxisListType.X,
                                op=ALU.add)
                        else:
                            sq_scr = p1t.tile([128, 128], F32, name="sq_scr")
                            for h in range(4):
                                nc.scalar.activation(
                                    sq_scr[:], qk_ps[:, h * 128:(h + 1) * 128],
                                    AF.Square, accum_out=ss[:, h:h + 1])
                        # ---- stage qk to SBUF in f32 (frees the PSUM bank
                        # early; single bf16 rounding happens at qrot) ----
                        qksb = p1t.tile([128, 4, 128], F32, name="qksb")
                        nc.scalar.copy(qksb[:],
                                       qk_ps[:].rearrange("p (h d) -> p h d", h=4))
                        h1, h2 = qksb[:, :, 0:64], qksb[:, :, 64:128]
                        cos_b = _bcast_mid(cos_sb[:, t, :], 4)
                        sin_b = _bcast_mid(sin_sb[:, t, :], 4)
                        ra = p1t.tile([128, 4, 64], F32, name="ra")
                        rb = p1t.tile([128, 4, 64], F32, name="rb")
                        rot = p1t.tile([128, 4, 128], F32, name="rot")
                        nc.vector.tensor_mul(ra[:], h1, cos_b)
                        nc.vector.tensor_mul(rb[:], h2, sin_b)
                        nc.vector.tensor_add(rot[:, :, 0:64], ra[:], rb[:])
                        nc.vector.tensor_mul(ra[:], h2, cos_b)
                        nc.vector.tensor_mul(rb[:], h1, sin_b)
                        nc.vector.tensor_sub(rot[:, :, 64:128], ra[:], rb[:])
                        rstd = _rsqrt_dve(nc, p1t, ss[:], 4, HD, "rq", iters=2)
                        qrot = p1t.tile([128, 4, 128], BF16, name="qrot")
                        for h in range(4):
                            nc.vector.tensor_scalar_mul(qrot[:, h, :], in0=rot[:, h, :],
                                                        scalar1=rstd[:, h:h + 1])
                        if pending_tp is not None:
                            if t == tt - 1:
                                # tile-14's transposes also via DMA: its ACT
                                # copyback would sit ahead of P2's first exps
                                pt14, pq14 = pending_tp
                                for h in range(4):
                                    nc.sync.dma_start_transpose(
                                        qkT[:, pt14, h, :], pq14[:, h, :])
                            else:
                                p1_transp(*pending_tp)
                        pending_tp = (t, qrot)
                        if vg_ps is not None:
                            if pending_vg is not None:
                                p1_vg(*pending_vg)
                            pending_vg = (t, vg_ps)

                    pend_corr = []
                    for t in range(tt):
                        s, half = t // 2, t % 2
                        if half == 0 and (b, s) not in xT_pre:
                            load_super(b, s)
                        xT_t = xT_pre[(b, s)]
                        if half == 1:
                            del xT_pre[(b, s)]
                            # prefetch 3 supers ahead
                            if s + 3 < tt // 2 and (b, s + 3) not in xT_pre:
                                load_super(b, s + 3)
                        xsl = slice(half * 128, half * 128 + 128)
                        qk_ps = mm_ps.tile([128, 512], F32, name="qk_ps")
                        vg_ps = mm_ps.tile([128, 512], F32, name="vg_ps")
                        if b == 0 and t < 1:
                            # warmup: corrections need the W lo-planes, which
                            # are still streaming in -- run tiles 0-2's mains
                            # (hi-only) first so the PE isn't DMA-gated
                            p1_main(xT_t, xsl, qk_ps, slice(0, 512))
                            p1_main(xT_t, xsl, vg_ps, slice(512, 1024))
                            pend_corr.append((t, xT_t, xsl, qk_ps, vg_ps))
                            continue
                        if pend_corr:
                            p1_main(xT_t, xsl, qk_ps, slice(0, 512))
                            p1_main(xT_t, xsl, vg_ps, slice(512, 1024))
                            for pt, pxT, pxsl, pqk, pvg in pend_corr:
                                p1_corr(pxT, pxsl, pqk, slice(0, 512))
                                p1_corr(pxT, pxsl, pvg, slice(512, 1024))
                                p1_post(pt, pqk, pvg)
                            pend_corr = []
                            p1_corr(xT_t, xsl, qk_ps, slice(0, 512))
                            p1_corr(xT_t, xsl, vg_ps, slice(512, 1024))
                            p1_post(t, qk_ps, vg_ps)
                            continue
                        p1_main(xT_t, xsl, qk_ps, slice(0, 512))
                        p1_corr(xT_t, xsl, qk_ps, slice(0, 512))
                        p1_main(xT_t, xsl, vg_ps, slice(512, 1024))
                        p1_corr(xT_t, xsl, vg_ps, slice(512, 1024))
                        p1_post(t, qk_ps, vg_ps)
                    # vg first: its ACT copies are ready immediately and
                    # must not queue behind P2's first exps. The LAST tile's
                    # transposes go on the DMA xbar: a PE transpose would
                    # sit in the in-order PE stream waiting ~2.4us for the
                    # rope chain, stalling P2's first scores; its qkT slice
                    # is only read by chunk 7, ~40us later.
                    # the last tile's SiLU chain is deferred into P2 (after
                    # the first exp groups) so only its v/g copies sit ahead
                    # of the exps on ACT; its g_sb slice is read at chunk 7
                    pending_silu = p1_vg(*pending_vg, pool=p2s,
                                         defer_silu=True)
                    pending_vg = None
                    lt, lqrot = pending_tp
                    pending_tp = None
                    for h in range(4):
                        nc.sync.dma_start_transpose(qkT[:, lt, h, :],
                                                    lqrot[:, h, :])
                # prefetch next batch's first supers during phase 2
                if b + 1 < nb:
                    for s in range(2):
                        load_super(b + 1, s)
                if phases < 2:
                    if pending_silu is not None:
                        pending_silu(bufs=1)
                        pending_silu = None
                    with tc.tile_pool(name="dump", bufs=2) as dump:
                        for t in range(tt):
                            d_t = dump.tile([128, 256], F32, name="d_t")
                            nc.vector.tensor_copy(d_t[:], v_sb[:, t, 0:256])
                            nc.vector.tensor_add(d_t[:], d_t[:], g_sb[:, t, :])
                            nc.sync.dma_start(
                                y_d[b, t * 128:(t + 1) * 128, :], d_t[:])
                    continue
                # ================= Phase 2 =================
                with tc.tile_pool(name="sc_ps", bufs=2, space="PSUM") as sc_ps, \
                     tc.tile_pool(name="av_ps", bufs=4, space="PSUM") as av_ps:
                    # --- job list: groups of <=2 score pairs; diagonal is its
                    # own group (needs the causal mask) -------------------
                    groups = []
                    for c in range(nch):
                        per_var = []
                        for var in range(2):
                            gs = []
                            prs_all = list(range(c + 1))
                            for i in range(0, len(prs_all), 2):
                                grp = prs_all[i:i + 2]
                                gs.append((c, var, grp, c in grp))
                            per_var.append(gs)
                        # interleave var streams; keep var0's diag before
                        # var1's diag so the pre-epilogue still leads
                        n = len(per_var[0])
                        for i in range(n):
                            groups.append(per_var[0][i])
                            groups.append(per_var[1][i])

                    sc_tiles = {}

                    def emit_sc(gi):
                        c, var, prs, diag = groups[gi]
                        scp = sc_ps.tile([128, 4, CH], F32, name="sc")
                        qch = qkT[:, 2 * c:2 * c + 2, var, :]
                        for pi, jp in enumerate(prs):
                            for jj in range(2):
                                nc.tensor.matmul(
                                    scp[:, 2 * pi + jj, :],
                                    qkT[:, 2 * jp + jj, 2 + var, :],
                                    qch, start=True, stop=True)
                        sc_tiles[gi] = scp

                    emit_sc(0)
                    yps = {}
                    for gi, (c, var, prs, diag) in enumerate(groups):
                        if var == 0 and prs[0] == 0:
                            for v2 in range(2):
                                for m in range(2):
                                    yps[(v2, m)] = av_ps.tile([128, 258], F32,
                                                              name="yacc")
                        n = 2 * len(prs)
                        scp = sc_tiles.pop(gi)
                        probs = p2s.tile([128, 4, CH], F32R, name="probs")
                        nc.scalar.activation(probs[:, 0:n, :], scp[:, 0:n, :],
                                             AF.Exp, scale=SCALE)
                        if diag:
                            pi = prs.index(c)
                            nc.vector.tensor_mul(
                                probs[:, 2 * pi:2 * pi + 2, :],
                                probs[:, 2 * pi:2 * pi + 2, :], mask_sb[:])
                        # emit next group's scores ahead of this group's AV
                        if gi + 1 < len(groups):
                            emit_sc(gi + 1)
                        if gi == 1 and pending_silu is not None:
                            pending_silu(bufs=1)
                            pending_silu = None
                        for pi, jp in enumerate(prs):
                            for jj in range(2):
                                j = 2 * jp + jj
                                for m in range(2):
                                    if j == 2 * c + 1 and m == 0:
                                        # fully-masked diagonal block: probs
                                        # are exactly zero there -> skip
                                        continue
                                    nc.tensor.matmul(
                                        yps[(var, m)][:],
                                        probs[:, 2 * pi + jj, m * 128:(m + 1) * 128],
                                        v_sb[:, j, 0:258],
                                        start=(j == 0),
                                        stop=(j == 2 * c + 1 - (1 - m)))
                        if diag and var == 0:
                            # var0 accumulators are complete: start the
                            # normalize of y1 while var1's attention runs
                            pre_ep = {}
                            for m in range(2):
                                y1p = yps[(0, m)]
                                r1 = p2e.tile([128, 1], F32, name="r1")
                                nc.vector.reciprocal(r1[:], y1p[:, 256:257])
                                t1 = p2e.tile([128, 256], F32, name="t1")
                                nc.vector.tensor_scalar_mul(
                                    t1[:], in0=y1p[:, 0:256], scalar1=r1[:])
                                pre_ep[m] = t1
                        if not (diag and var == 1):
                            continue
                        # ---- epilogue for chunk c ----
                        ssy = p2e.tile([128, 2], F32, name="ssy")
                        ygs = []
                        for m in range(2):
                            y2p = yps[(1, m)]
                            # v col 257 = -1/lam -> r2n is one recip away
                            r2n = p2e.tile([128, 1], F32, name="r2n")
                            nc.vector.reciprocal(r2n[:], y2p[:, 257:258])
                            t1 = pre_ep[m]
                            yt = p2e.tile([128, 256], F32, name="yt")
                            nc.vector.scalar_tensor_tensor(
                                yt[:], y2p[:, 0:256], r2n[:], t1[:],
                                op0=ALU.mult, op1=ALU.add)
                            yg = p2e.tile([128, 256], F32, name="yg", bufs=2)
                            nc.vector.tensor_mul(yg[:], yt[:],
                                                 g_sb[:, 2 * c + m, :])
                            if c == nch - 1 and m == 0:
                                # tail chunk: m=0 stats on the otherwise-idle
                                # ACT, m=1 on DVE -- the two run in parallel
                                sq = p2e.tile([128, 256], F32, name="sq2")
                                nc.scalar.activation(sq[:], yg[:], AF.Square,
                                                     accum_out=ssy[:, m:m + 1])
                            else:
                                sq = p2e.tile([128, 256], F32, name="sq2")
                                nc.vector.tensor_mul(sq[:], yg[:], yg[:])
                                nc.vector.tensor_reduce(
                                    ssy[:, m:m + 1], sq[:],
                                    axis=mybir.AxisListType.X, op=ALU.add)
                            ygs.append(yg)
                        # rsy absorbs the (1-lambda_init) factor:
                        # (ms/C^2)^-0.5 = C * ms^-0.5. The tail chunk drops
                        # to 1 Newton iter (<=1.7e-3 rel on 256 tokens) to
                        # shorten the end-of-kernel drain chain.
                        CI2 = 1.0 / (ONE_MINUS_LI * ONE_MINUS_LI)
                        rsy = _rsqrt_dve(nc, p2e, ssy[:], 2, 256 / CI2, "ry",
                                         iters=1 if c == nch - 1 else 2,
                                         eps=EPS * CI2)
                        out_t = p2e.tile([128, 2, 256], F32, name="out_t",
                                         bufs=2)
                        for m in range(2):
                            nc.vector.tensor_scalar_mul(
                                out_t[:, m, :], in0=ygs[m][:],
                                scalar1=rsy[:, m:m + 1])
                            if c == nch - 1:
                                # tail: per-half DMA so the first issue
                                # overlaps the second half's scale
                                nc.sync.dma_start(
                                    y_d[b, (2 * c + m) * 128:
                                        (2 * c + m + 1) * 128, :],
                                    out_t[:, m, :])
                        if c != nch - 1:
                            nc.sync.dma_start(
                                y_d[b, 2 * c * 128:(2 * c + 2) * 128, :]
                                .rearrange("(m p) c -> p m c", p=128),
                                out_t[:])
    nc.compile()
    return nc


_NC = None


def prep_in_maps(hidden_states, W_qkv, lambda_q1, lambda_k1, lambda_q2,
                 lambda_k2, W_g):
    import ml_dtypes
    bf16 = ml_dtypes.bfloat16
    f8 = ml_dtypes.float8_e4m3
    x = np.asarray(hidden_states, dtype=np.float32)
    xt = np.ascontiguousarray(x.transpose(0, 2, 1))        # [B, D, T] f32
    x_hi = xt.astype(f8)
    x_lo = (xt - x_hi.astype(np.float32)).astype(f8)
    # pack [B, D, S, 2, 256]: hi and lo planes adjacent per 256-tok super
    xt_p = np.empty((B, D, T // 256, 2, 256), dtype=f8)
    xt_p[:, :, :, 0, :] = x_hi.reshape(B, D, T // 256, 256)
    xt_p[:, :, :, 1, :] = x_lo.reshape(B, D, T // 256, 256)
    W_qkv = np.asarray(W_qkv, dtype=np.float32)
    W_g = np.asarray(W_g, dtype=np.float32)

    t_ar = np.arange(T, dtype=np.float32)
    inv_freq = (1.0 / 10000.0 ** (np.arange(0, HD, 2, dtype=np.float32) / HD)
                ).astype(np.float32)
    freqs = np.outer(t_ar, inv_freq).astype(np.float32)
    cos = np.cos(freqs).astype(np.float32)
    sin = np.sin(freqs).astype(np.float32)

    # multiplicative 0/1 causal mask (applied to probs AFTER exp)
    masks = np.empty((128, 2, CH), dtype=np.float32)
    kk = np.arange(128)[:, None]
    qq = np.arange(CH)[None, :]
    for m in range(2):
        masks[:, m, :] = np.where(m * 128 + kk <= qq, 1.0, 0.0)
    
    ident = np.eye(128, dtype=bf16)

    lam1 = np.exp(np.sum(np.asarray(lambda_q1, np.float32)
                         * np.asarray(lambda_k1, np.float32), axis=-1))
    lam2 = np.exp(np.sum(np.asarray(lambda_q2, np.float32)
                         * np.asarray(lambda_k2, np.float32), axis=-1))
    lam = (lam1 - lam2 + LAMBDA_INIT).astype(np.float32)   # [8]

    in_maps = []
    for c in range(N_CORES):
        base = 2 * c * 384
        w_cols = [
            W_qkv[:, base:base + 128],            # q1
            W_qkv[:, base + 384:base + 512],      # q2
            W_qkv[:, base + 128:base + 256],      # k1
            W_qkv[:, base + 512:base + 640],      # k2
            W_qkv[:, base + 256:base + 384],      # v1
            W_qkv[:, base + 640:base + 768],      # v2
            W_g[:, c * 256:(c + 1) * 256],        # gate
        ]
        wc = np.concatenate(w_cols, axis=1) * WSCALE        # [D, 1024] f32
        w_hi = wc.astype(f8)
        w_lo = (wc - w_hi.astype(np.float32)).astype(f8)
        # pack [D, 2, 1024]: slot0 = LO, slot1 = HI (correction AP order)
        wcat = np.empty((D, 2, 1024), dtype=f8)
        wcat[:, 0, :] = w_lo
        wcat[:, 1, :] = w_hi
        ones = np.zeros((128, 4), dtype=np.float32)
        ones[:, 0] = 1.0
        ones[:, 1] = -1.0 / lam[c]
        in_maps.append({
            "xt": xt_p[:, :, :, :, :], "wcat": wcat, "cos": cos, "sin": sin,
            "masks": masks, "ident": ident, "ones": ones,
        })

    return in_maps


def kernel(hidden_states, W_qkv, lambda_q1, lambda_k1, lambda_q2, lambda_k2,
           W_g, **run_kwargs):
    global _NC
    if _NC is None:
        _NC = build()
    in_maps = prep_in_maps(hidden_states, W_qkv, lambda_q1, lambda_k1,
                           lambda_q2, lambda_k2, W_g)
    res = run_bass_kernel_spmd(_NC, in_maps, core_ids=list(range(N_CORES)),
                               **run_kwargs)
    out = np.empty((B, T, D), dtype=np.float32)
    for c in range(N_CORES):
        out[:, :, c * 256:(c + 1) * 256] = res.results[c]["y"]
    if run_kwargs:
        return out, res
    return out



# revision 58
# speedup vs baseline: 1.0053x; 1.0020x over previous
"""MixerDiffAttention Trainium2 kernel (v4 — fp8 DoubleRow projection).

Sharding: 8 cores = 8 head-pairs (tensor parallel over head-pair dim).
Each core processes BOTH batches for its head-pair: the per-core weight
slice (768 qkv cols + 256 gate cols) stays SBUF-resident, and each core
produces the disjoint output slice y[:, :, hp*256:(hp+1)*256].

Key scheduling facts (from the timeline cost model): matmul cost =
out_free_size x dtype_rate (contraction depth is free; fp8e4 DoubleRow
runs at 0.5 cycles/row AND contracts 2x128 rows per instruction),
engines execute in per-engine program order, any PE idle resets the
p-state ramp (next 3us at 2x cycle time), and reopened tile pools
carry WAR deps on the previous scope's readers -- so the P2 SBUF pools
are hoisted to the outer scope. q/k transposes run on the PE (identity
matmul, 53ns) deferred one tile behind the rope chain; all-DMA-xbar
transposes saturate SP.SEQ's ~0.4us-per-issue in-order path and stall
P2 by ~10us, but the LAST tile's go via DMA so the PE stream flows
straight into P2's scores.

Projection precision: x and 64*W are split host-side into fp8(e4m3)
hi+lo pairs (hi = fp8(a), lo = fp8(a - hi)). z = x_hi@W_hi +
(x_hi@W_lo + x_lo@W_hi), the correction pair riding the two DoubleRow
slots of one instruction per 128-feature block; the dropped lo@lo term
is ~0.13%. Measured on the real inputs this is MORE accurate than the
bf16 path (proj rms 1.2e-3 vs 2.4e-3) at 0.75x the PE cost. The
uniform 64x output scale is absorbed by the q/k RMSNorm, the final
group RMSNorm (for v and the gate product), and an exp scale of -1/64
in the SiLU sigmoid.

Per core, per batch:
  Phase 1 (per 128-token tile; x and W stream in as packed fp8 hi|lo,
    256-token 512B-run DMAs; tile 0's corrections deferred behind tile
    1's hi-only mains so the warmup isn't gated on the W lo-planes):
    qk projection matmuls first, then v|gate (the qk-stats chain starts
    half a tile early); qk staged to SBUF f32 (frees the PSUM bank);
    RMSNorm stats via ACT Square+accum; rstd via DVE Quake-seed Newton
    (2 it); RoPE on DVE in f32; single bf16 rounding at the rstd-scale;
    feature-major q/k via deferred PE transposes + ACT copyback; v
    (+ones column for softmax row sums) and raw gate copied by ACT one
    tile late; SiLU gate via sigma=1/(1+exp(-g)): ACT Exp, Pool add,
    DVE recip-approx, Pool mult (all off the critical chain).
  Phase 2 (flat software-pipelined group stream): score matmuls for
    group g+1 are emitted BEFORE the AV matmuls of group g, so the PE
    never sits behind ACT's exp. Exps are batched 2 score-pairs per ACT
    instruction (exp_and_friends table set only -> no table swaps); the
    causal-diagonal slice is masked multiplicatively after exp (exact
    0/1 f32 on DVE); the diagonal AV block that is fully causal-masked
    is skipped outright. The epilogue overlaps attention: y1's normalize
    starts when var0's accumulators finish; the diff combine, SiLU
    gating, and group RMSNorm (rsqrt absorbs the 1-lambda_init factor)
    finish after var1, with sum-of-squares on DVE (ACT on the tail
    chunk where ACT is idle).
"""
import sys
sys.path.insert(0, "/opt/trn_rl_repo")
import numpy as np
import concourse.bass as bass
from concourse import bacc
import concourse.tile as tile
from concourse import mybir
from concourse.bass_utils import run_bass_kernel_spmd

F32 = mybir.dt.float32
F32R = mybir.dt.float32r
BF16 = mybir.dt.bfloat16
F8 = mybir.dt.float8e4
DR = mybir.MatmulPerfMode.DoubleRow
AF = mybir.ActivationFunctionType
ALU = mybir.AluOpType
WSCALE = 64.0

B, T, D, HD = 2, 2048, 2048, 128
KT = D // 128          # 16 contraction tiles
TT = T // 128          # 16 token tiles
CH = 256               # query-chunk width in phase 2
NCH = T // CH          # 8 chunks
N_CORES = 8
LAMBDA_INIT = 0.8 - 0.6 * float(np.exp(-0.3 * 6))
ONE_MINUS_LI = 1.0 - LAMBDA_INIT
SCALE = float(HD ** -0.5)
EPS = 1e-6


def _bcast_mid(ap, n):
    # [P, F] AP -> [P, n, F] with a zero-stride middle dim
    return bass.AP(tensor=ap.tensor, offset=ap.offset,
                   ap=[ap.ap[0], [0, n], *ap.ap[1:]])


def _rsqrt_dve(nc, pool, ss_ap, width, mean_div, tag, iters=2, eps=EPS):
    """rstd = (ss/mean_div + EPS) ** -0.5 entirely on DVE.

    Quake-III bit-trick seed + Newton iterations (2 it: ~5e-6 rel err;
    1 it: ~1.7e-3 max rel err); avoids ACT Ln/Sqrt so the whole kernel
    stays inside one ACT table set."""
    I32 = mybir.dt.int32
    ms = pool.tile([128, width], F32, name=tag + "_ms")
    nc.vector.tensor_scalar(out=ms[:], in0=ss_ap, scalar1=1.0 / mean_div,
                            scalar2=eps, op0=ALU.mult, op1=ALU.add)
    iv = pool.tile([128, width], I32, name=tag + "_iv")
    nc.vector.tensor_scalar(out=iv[:], in0=ms[:].bitcast(I32), scalar1=1,
                            scalar2=None, op0=ALU.logical_shift_right)
    y = pool.tile([128, width], F32, name=tag + "_y")
    nc.vector.tensor_scalar(out=y[:].bitcast(I32), in0=iv[:], scalar1=-1,
                            scalar2=0x5F3759DF, op0=ALU.mult, op1=ALU.add)
    a = pool.tile([128, width], F32, name=tag + "_a")
    u = pool.tile([128, width], F32, name=tag + "_u")
    for _ in range(iters):
        nc.vector.tensor_mul(a[:], y[:], y[:])
        nc.vector.tensor_mul(a[:], a[:], ms[:])
        nc.vector.tensor_scalar(out=u[:], in0=a[:], scalar1=-0.5, scalar2=1.5,
                                op0=ALU.mult, op1=ALU.add)
        nc.vector.tensor_mul(y[:], y[:], u[:])
    return y


def build(tt=TT, nb=B, phases=2):
    nch = tt * 128 // CH
    nc = bacc.Bacc("TRN2", target_bir_lowering=False, debug=False,
                   num_devices=N_CORES)
    # x / W in fp8 hi+lo pairs: x packed [D, S, 2(hi,lo), 256] so one
    # 512B-run DMA per feature row pulls both planes of a super tile;
    # W packed [D, 2(lo,hi), 1024] so the correction matmul's moving AP
    # [lo|hi] pairs against the stationary x [hi|lo] DoubleRow slots.
    xt_d = nc.dram_tensor("xt", [nb, D, tt * 128 // 256, 2, 256], F8,
                          kind="ExternalInput").ap()
    w_d = nc.dram_tensor("wcat", [D, 2, 1024], F8, kind="ExternalInput").ap()
    cos_d = nc.dram_tensor("cos", [tt * 128, 64], F32, kind="ExternalInput").ap()
    sin_d = nc.dram_tensor("sin", [tt * 128, 64], F32, kind="ExternalInput").ap()
    mask_d = nc.dram_tensor("masks", [128, 2, CH], BF16, kind="ExternalInput").ap()
    id_d = nc.dram_tensor("ident", [128, 128], BF16, kind="ExternalInput").ap()
    ones_d = nc.dram_tensor("ones", [128, 4], BF16, kind="ExternalInput").ap()
    y_d = nc.dram_tensor("y", [nb, tt * 128, 256], F32, kind="ExternalOutput").ap()

    with tile.TileContext(nc) as tc:
        with tc.tile_pool(name="bigs", bufs=1) as bigs, \
             tc.tile_pool(name="consts", bufs=1) as consts, \
             tc.tile_pool(name="p2s", bufs=5) as p2s, \
             tc.tile_pool(name="p2e", bufs=5) as p2e, \
             tc.tile_pool(name="xtp", bufs=5) as xtp:
            # ---- weights first: the k=0..1 slices gate the first matmul ----
            wcat = bigs.tile([128, KT, 2, 1024], F8)
            w_v = w_d.rearrange("(k p) i c -> p k i c", p=128)

            # ---- 256-token "super tile" loads (fp8 hi+lo, 512B runs) ----
            xT_pre = {}

            def load_super(b, s):
                xv = xt_d[b].rearrange("(k p) s i t -> p k s i t", p=128)
                xp = xtp.tile([128, KT, 2, 256], F8, name="xT_s")
                for kh in range(2):
                    nc.sync.dma_start(
                        xp[:, kh * 8:(kh + 1) * 8, :, :],
                        xv[:, kh * 8:(kh + 1) * 8, s, :, :])
                xT_pre[(b, s)] = xp

            # first x blocks + W hi-planes gate the first (main-term)
            # matmuls; W lo-planes are only needed by the corrections,
            # which are deferred for the first two tiles (see p1 loop) --
            # so hi loads lead and the PE starts within ~2us
            xv0 = xt_d[0].rearrange("(k p) s i t -> p k s i t", p=128)
            xp0 = xtp.tile([128, KT, 2, 256], F8, name="xT_s")
            nc.sync.dma_start(xp0[:, 0:2, :, :], xv0[:, 0:2, 0, :, :])
            nc.sync.dma_start(wcat[:, 0:2, 1, :], w_v[:, 0:2, 1, :])
            nc.sync.dma_start(xp0[:, 2:8, :, :], xv0[:, 2:8, 0, :, :])
            nc.sync.dma_start(wcat[:, 2:8, 1, :], w_v[:, 2:8, 1, :])
            nc.sync.dma_start(xp0[:, 8:16, :, :], xv0[:, 8:16, 0, :, :])
            nc.sync.dma_start(wcat[:, 8:16, 1, :], w_v[:, 8:16, 1, :])
            xT_pre[(0, 0)] = xp0
            nc.sync.dma_start(wcat[:, 0:8, 0, :], w_v[:, 0:8, 0, :])
            nc.sync.dma_start(wcat[:, 8:16, 0, :], w_v[:, 8:16, 0, :])
            load_super(0, 1)
            # ---- small constants (needed only after the first projection) ----
            cos_sb = consts.tile([128, tt, 64], F32)
            nc.sync.dma_start(cos_sb[:], cos_d.rearrange("(t p) f -> p t f", p=128))
            sin_sb = consts.tile([128, tt, 64], F32)
            nc.sync.dma_start(sin_sb[:], sin_d.rearrange("(t p) f -> p t f", p=128))
            id_sb = consts.tile([128, 128], BF16)
            nc.sync.dma_start(id_sb[:], id_d)
            load_super(0, 2)
            mask_sb = consts.tile([128, 2, CH], BF16)
            nc.sync.dma_start(mask_sb[:], mask_d)
            ones_sb = consts.tile([128, 4], BF16)
            nc.sync.dma_start(ones_sb[:], ones_d)
            load_super(0, 3)

            # ---- per-batch persistent (reused sequentially) ----
            qkT = bigs.tile([128, tt, 4, 128], BF16)    # t-major; rows q1,q2,k1,k2
            v_sb = bigs.tile([128, tt, 260], BF16)      # [tok, v(256)|1|0 pad]
            g_sb = bigs.tile([128, tt, 256], F32)       # gate (raw -> silu'd JIT)

            for b in range(nb):
                # ================= Phase 1 =================
                with tc.tile_pool(name="p1t", bufs=3) as p1t, \
                     tc.tile_pool(name="mm_ps", bufs=3, space="PSUM") as mm_ps, \
                     tc.tile_pool(name="tp_ps", bufs=2, space="PSUM") as tp_ps:
                    # ones column for every tile in one strided write
                    nc.vector.tensor_copy(v_sb[:, :, 256:260],
                                          _bcast_mid(ones_sb[:], tt))

                    # q/k transposes on the PE (identity matmul, 53ns each)
                    # with an ACT copyback: all-DMA transposes saturate
                    # SP.SEQ's in-order ~0.4us-per-issue path and drain so
                    # late that P2(b0) stalls ~10us. Deferred one tile so
                    # the PE never waits on the rope chain.
                    def p1_transp(t, qrot):
                        tp = tp_ps.tile([128, 4, 128], BF16, name="tp")
                        for h in range(4):
                            nc.tensor.matmul(tp[:, h, :], qrot[:, h, :],
                                             id_sb[:], is_transpose=True)
                        nc.scalar.copy(qkT[:, t, :, :], tp[:])

                    pending_vg = None
                    pending_tp = None

                    def p1_vg(t, vg_ps, pool=None, defer_silu=False):
                        # v / raw gate copies + SiLU gate; deferred one tile
                        # so the next tile's squares lead the ACT queue
                        pool = pool or p1t
                        nc.scalar.copy(v_sb[:, t, 0:256], vg_ps[:, 0:256])
                        nc.scalar.copy(g_sb[:, t, :], vg_ps[:, 256:512])

                        def silu(bufs=3):
                            ge = pool.tile([128, 256], F32, name="ge",
                                           bufs=bufs)
                            # g_sb holds 64*z; sigmoid wants exp(-z)
                            nc.scalar.activation(ge[:], g_sb[:, t, :], AF.Exp,
                                                 scale=-1.0 / WSCALE)
                            gd = pool.tile([128, 256], F32, name="gd",
                                           bufs=bufs)
                            nc.gpsimd.tensor_scalar(out=gd[:], in0=ge[:],
                                                    scalar1=1.0, scalar2=None,
                                                    op0=ALU.add)
                            gr = pool.tile([128, 256], F32, name="gr",
                                           bufs=bufs)
                            nc.vector.reciprocal_approx_fast(out=gr[:],
                                                             in_=gd[:])
                            nc.gpsimd.tensor_mul(g_sb[:, t, :], g_sb[:, t, :],
                                                 gr[:])

                        if defer_silu:
                            return silu
                        silu()

                    def p1_main(xT_t, xsl, ps, cols):
                        # main term: x_hi @ W_hi, 2 k-blocks per inst
                        for r in range(KT // 2):
                            nc.tensor.matmul(
                                ps[:], xT_t[:, 2 * r:2 * r + 2, 0, xsl],
                                wcat[:, 2 * r:2 * r + 2, 1, cols],
                                start=(r == 0), stop=False, perf_mode=DR)

                    def p1_corr(xT_t, xsl, ps, cols):
                        # correction: x_hi@W_lo + x_lo@W_hi via the two
                        # DoubleRow slots of one inst per k-block
                        for r in range(KT):
                            nc.tensor.matmul(
                                ps[:], xT_t[:, r, :, xsl], wcat[:, r, :, cols],
                                start=False, stop=(r == KT - 1), perf_mode=DR)

                    def p1_post(t, qk_ps, vg_ps, last=False):
                        nonlocal pending_tp, pending_vg
                        # ---- q/k rmsnorm stats FIRST on ACT (they gate the
                        # rsqrt -> qrot -> transpose chain). For the LAST
                        # tile they move to DVE: its rope chain feeds only
                        # the chunk-7 DMA transpose (~40us of slack), and
                        # clearing ACT lets P2's first exps start ~1us
                        # earlier ----
                        ss = p1t.tile([128, 4], F32, name="ss")
                        if last:
                            sqv = p1t.tile([128, 4, 128], F32, name="sqv",
                                           bufs=1)
                            qkv = qk_ps[:].rearrange("p (h d) -> p h d", h=4)
                            nc.vector.tensor_mul(sqv[:], qkv, qkv)
                            nc.vector.tensor_reduce(
                                ss[:], sqv[:], axis=mybir.AxisListType.X,
                                op=ALU.add)
                        else:
                            sq_scr = p1t.tile([128, 128], F32, name="sq_scr")
                            for h in range(4):
                                nc.scalar.activation(
                                    sq_scr[:], qk_ps[:, h * 128:(h + 1) * 128],
                                    AF.Square, accum_out=ss[:, h:h + 1])
                        # ---- stage qk to SBUF in f32 (frees the PSUM bank
                        # early; single bf16 rounding happens at qrot) ----
                        qksb = p1t.tile([128, 4, 128], F32, name="qksb")
                        nc.scalar.copy(qksb[:],
                                       qk_ps[:].rearrange("p (h d) -> p h d", h=4))
                        h1, h2 = qksb[:, :, 0:64], qksb[:, :, 64:128]
                        cos_b = _bcast_mid(cos_sb[:, t, :], 4)
                        sin_b = _bcast_mid(sin_sb[:, t, :], 4)
                        ra = p1t.tile([128, 4, 64], F32, name="ra")
                        rb = p1t.tile([128, 4, 64], F32, name="rb")
                        rot = p1t.tile([128, 4, 128], F32, name="rot")
                        nc.vector.tensor_mul(ra[:], h1, cos_b)
                        nc.vector.tensor_mul(rb[:], h2, sin_b)
                        nc.vector.tensor_add(rot[:, :, 0:64], ra[:], rb[:])
                        nc.vector.tensor_mul(ra[:], h2, cos_b)
                        nc.vector.tensor_mul(rb[:], h1, sin_b)
                        nc.vector.tensor_sub(rot[:, :, 64:128], ra[:], rb[:])
                        rstd = _rsqrt_dve(nc, p1t, ss[:], 4, HD, "rq", iters=2)
                        qrot = p1t.tile([128, 4, 128], BF16, name="qrot")
                        for h in range(4):
                            nc.vector.tensor_scalar_mul(qrot[:, h, :], in0=rot[:, h, :],
                                                        scalar1=rstd[:, h:h + 1])
                        if pending_tp is not None:
                            if t == tt - 1:
                                # tile-14's transposes also via DMA: its ACT
                                # copyback would sit ahead of P2's first exps
                                pt14, pq14 = pending_tp
                                for h in range(4):
                                    nc.sync.dma_start_transpose(
                                        qkT[:, pt14, h, :], pq14[:, h, :])
                            else:
                                p1_transp(*pending_tp)
                        pending_tp = (t, qrot)
                        if vg_ps is not None:
                            if pending_vg is not None:
                                p1_vg(*pending_vg)
                            pending_vg = (t, vg_ps)

                    pend_corr = []
                    for t in range(tt):
                        s, half = t // 2, t % 2
                        if half == 0 and (b, s) not in xT_pre:
                            load_super(b, s)
                        xT_t = xT_pre[(b, s)]
                        if half == 1:
                            del xT_pre[(b, s)]
                            # prefetch 3 supers ahead
                            if s + 3 < tt // 2 and (b, s + 3) not in xT_pre:
                                load_super(b, s + 3)
                        xsl = slice(half * 128, half * 128 + 128)
                        qk_ps = mm_ps.tile([128, 512], F32, name="qk_ps")
                        vg_ps = mm_ps.tile([128, 512], F32, name="vg_ps")
                        if b == 0 and t < 1:
                            # warmup: corrections need the W lo-planes, which
                            # are still streaming in -- run tiles 0-2's mains
                            # (hi-only) first so the PE isn't DMA-gated
                            p1_main(xT_t, xsl, qk_ps, slice(0, 512))
                            p1_main(xT_t, xsl, vg_ps, slice(512, 1024))
                            pend_corr.append((t, xT_t, xsl, qk_ps, vg_ps))
                            continue
                        if pend_corr:
                            p1_main(xT_t, xsl, qk_ps, slice(0, 512))
                            p1_main(xT_t, xsl, vg_ps, slice(512, 1024))
                            for pt, pxT, pxsl, pqk, pvg in pend_corr:
                                p1_corr(pxT, pxsl, pqk, slice(0, 512))
                                p1_corr(pxT, pxsl, pvg, slice(512, 1024))
                                p1_post(pt, pqk, pvg)
                            pend_corr = []
                            p1_corr(xT_t, xsl, qk_ps, slice(0, 512))
                            p1_corr(xT_t, xsl, vg_ps, slice(512, 1024))
                            p1_post(t, qk_ps, vg_ps)
                            continue
                        p1_main(xT_t, xsl, qk_ps, slice(0, 512))
                        p1_corr(xT_t, xsl, qk_ps, slice(0, 512))
                        p1_main(xT_t, xsl, vg_ps, slice(512, 1024))
                        p1_corr(xT_t, xsl, vg_ps, slice(512, 1024))
                        p1_post(t, qk_ps, vg_ps)
                    # vg first: its ACT copies are ready immediately and
                    # must not queue behind P2's first exps. The LAST tile's
                    # transposes go on the DMA xbar: a PE transpose would
                    # sit in the in-order PE stream waiting ~2.4us for the
                    # rope chain, stalling P2's first scores; its qkT slice
                    # is only read by chunk 7, ~40us later.
                    # the last tile's SiLU chain is deferred into P2 (after
                    # the first exp groups) so only its v/g copies sit ahead
                    # of the exps on ACT; its g_sb slice is read at chunk 7
                    pending_silu = p1_vg(*pending_vg, pool=p2s,
                                         defer_silu=True)
                    pending_vg = None
                    lt, lqrot = pending_tp
                    pending_tp = None
                    for h in range(4):
                        nc.sync.dma_start_transpose(qkT[:, lt, h, :],
                                                    lqrot[:, h, :])
                # prefetch next batch's first supers during phase 2
                if b + 1 < nb:
                    for s in range(2):
                        load_super(b + 1, s)
                if phases < 2:
                    if pending_silu is not None:
                        pending_silu(bufs=1)
                        pending_silu = None
                    with tc.tile_pool(name="dump", bufs=2) as dump:
                        for t in range(tt):
                            d_t = dump.tile([128, 256], F32, name="d_t")
                            nc.vector.tensor_copy(d_t[:], v_sb[:, t, 0:256])
                            nc.vector.tensor_add(d_t[:], d_t[:], g_sb[:, t, :])
                            nc.sync.dma_start(
                                y_d[b, t * 128:(t + 1) * 128, :], d_t[:])
                    continue
                # ================= Phase 2 =================
                with tc.tile_pool(name="sc_ps", bufs=2, space="PSUM") as sc_ps, \
                     tc.tile_pool(name="av_ps", bufs=4, space="PSUM") as av_ps:
                    # --- job list: groups of <=2 score pairs; diagonal is its
                    # own group (needs the causal mask) -------------------
                    groups = []
                    for c in range(nch):
                        per_var = []
                        for var in range(2):
                            gs = []
                            prs_all = list(range(c + 1))
                            for i in range(0, len(prs_all), 2):
                                grp = prs_all[i:i + 2]
                                gs.append((c, var, grp, c in grp))
                            per_var.append(gs)
                        # interleave var streams; keep var0's diag before
                        # var1's diag so the pre-epilogue still leads
                        n = len(per_var[0])
                        for i in range(n):
                            groups.append(per_var[0][i])
                            groups.append(per_var[1][i])

                    sc_tiles = {}

                    def emit_sc(gi):
                        c, var, prs, diag = groups[gi]
                        scp = sc_ps.tile([128, 4, CH], F32, name="sc")
                        qch = qkT[:, 2 * c:2 * c + 2, var, :]
                        for pi, jp in enumerate(prs):
                            for jj in range(2):
                                nc.tensor.matmul(
                                    scp[:, 2 * pi + jj, :],
                                    qkT[:, 2 * jp + jj, 2 + var, :],
                                    qch, start=True, stop=True)
                        sc_tiles[gi] = scp

                    emit_sc(0)
                    yps = {}
                    for gi, (c, var, prs, diag) in enumerate(groups):
                        if var == 0 and prs[0] == 0:
                            for v2 in range(2):
                                for m in range(2):
                                    yps[(v2, m)] = av_ps.tile([128, 258], F32,
                                                              name="yacc")
                        n = 2 * len(prs)
                        scp = sc_tiles.pop(gi)
                        probs = p2s.tile([128, 4, CH], BF16, name="probs")
                        nc.scalar.activation(probs[:, 0:n, :], scp[:, 0:n, :],
                                             AF.Exp, scale=SCALE)
                        if diag:
                            pi = prs.index(c)
                            nc.vector.tensor_mul(
                                probs[:, 2 * pi:2 * pi + 2, :],
                                probs[:, 2 * pi:2 * pi + 2, :], mask_sb[:])
                        # emit next group's scores ahead of this group's AV
                        if gi + 1 < len(groups):
                            emit_sc(gi + 1)
                        if gi == 1 and pending_silu is not None:
                            pending_silu(bufs=1)
                            pending_silu = None
                        for pi, jp in enumerate(prs):
                            for jj in range(2):
                                j = 2 * jp + jj
                                for m in range(2):
                                    if j == 2 * c + 1 and m == 0:
                                        # fully-masked diagonal block: probs
                                        # are exactly zero there -> skip
                                        continue
                                    nc.tensor.matmul(
                                        yps[(var, m)][:],
                                        probs[:, 2 * pi + jj, m * 128:(m + 1) * 128],
                                        v_sb[:, j, 0:258],
                                        start=(j == 0),
                                        stop=(j == 2 * c + 1 - (1 - m)))
                        if diag and var == 0:
                            # var0 accumulators are complete: start the
                            # normalize of y1 while var1's attention runs
                            pre_ep = {}
                            for m in range(2):
                                y1p = yps[(0, m)]
                                r1 = p2e.tile([128, 1], F32, name="r1")
                                nc.vector.reciprocal(r1[:], y1p[:, 256:257])
                                t1 = p2e.tile([128, 256], F32, name="t1")
                                nc.vector.tensor_scalar_mul(
                                    t1[:], in0=y1p[:, 0:256], scalar1=r1[:])
                                pre_ep[m] = t1
                        if not (diag and var == 1):
                            continue
                        # ---- epilogue for chunk c ----
                        ssy = p2e.tile([128, 2], F32, name="ssy")
                        ygs = []
                        for m in range(2):
                            y2p = yps[(1, m)]
                            # v col 257 = -1/lam -> r2n is one recip away
                            r2n = p2e.tile([128, 1], F32, name="r2n")
                            nc.vector.reciprocal(r2n[:], y2p[:, 257:258])
                            t1 = pre_ep[m]
                            yt = p2e.tile([128, 256], F32, name="yt")
                            nc.vector.scalar_tensor_tensor(
                                yt[:], y2p[:, 0:256], r2n[:], t1[:],
                                op0=ALU.mult, op1=ALU.add)
                            yg = p2e.tile([128, 256], F32, name="yg", bufs=2)
                            nc.vector.tensor_mul(yg[:], yt[:],
                                                 g_sb[:, 2 * c + m, :])
                            if c == nch - 1 and m == 0:
                                # tail chunk: m=0 stats on the otherwise-idle
                                # ACT, m=1 on DVE -- the two run in parallel
                                sq = p2e.tile([128, 256], F32, name="sq2")
                                nc.scalar.activation(sq[:], yg[:], AF.Square,
                                                     accum_out=ssy[:, m:m + 1])
                            else:
                                sq = p2e.tile([128, 256], F32, name="sq2")
                                nc.vector.tensor_mul(sq[:], yg[:], yg[:])
                                nc.vector.tensor_reduce(
                                    ssy[:, m:m + 1], sq[:],
                                    axis=mybir.AxisListType.X, op=ALU.add)
                            ygs.append(yg)
                        # rsy absorbs the (1-lambda_init) factor:
                        # (ms/C^2)^-0.5 = C * ms^-0.5. The tail chunk drops
                        # to 1 Newton iter (<=1.7e-3 rel on 256 tokens) to
                        # shorten the end-of-kernel drain chain.
                        CI2 = 1.0 / (ONE_MINUS_LI * ONE_MINUS_LI)
                        rsy = _rsqrt_dve(nc, p2e, ssy[:], 2, 256 / CI2, "ry",
                                         iters=1 if c == nch - 1 else 2,
                                         eps=EPS * CI2)
                        out_t = p2e.tile([128, 2, 256], F32, name="out_t",
                                         bufs=2)
                        for m in range(2):
                            nc.vector.tensor_scalar_mul(
                                out_t[:, m, :], in0=ygs[m][:],
                                scalar1=rsy[:, m:m + 1])
                            if c == nch - 1:
                                # tail: per-half DMA so the first issue
                                # overlaps the second half's scale
                                nc.sync.dma_start(
                                    y_d[b, (2 * c + m) * 128:
                                        (2 * c + m + 1) * 128, :],
                                    out_t[:, m, :])
                        if c != nch - 1:
                            nc.sync.dma_start(
                                y_d[b, 2 * c * 128:(2 * c + 2) * 128, :]
                                .rearrange("(m p) c -> p m c", p=128),
                                out_t[:])
    nc.compile()
    return nc


_NC = None


def prep_in_maps(hidden_states, W_qkv, lambda_q1, lambda_k1, lambda_q2,
                 lambda_k2, W_g):
    import ml_dtypes
    bf16 = ml_dtypes.bfloat16
    f8 = ml_dtypes.float8_e4m3
    x = np.asarray(hidden_states, dtype=np.float32)
    xt = np.ascontiguousarray(x.transpose(0, 2, 1))        # [B, D, T] f32
    x_hi = xt.astype(f8)
    x_lo = (xt - x_hi.astype(np.float32)).astype(f8)
    # pack [B, D, S, 2, 256]: hi and lo planes adjacent per 256-tok super
    xt_p = np.empty((B, D, T // 256, 2, 256), dtype=f8)
    xt_p[:, :, :, 0, :] = x_hi.reshape(B, D, T // 256, 256)
    xt_p[:, :, :, 1, :] = x_lo.reshape(B, D, T // 256, 256)
    W_qkv = np.asarray(W_qkv, dtype=np.float32)
    W_g = np.asarray(W_g, dtype=np.float32)

    t_ar = np.arange(T, dtype=np.float32)
    inv_freq = (1.0 / 10000.0 ** (np.arange(0, HD, 2, dtype=np.float32) / HD)
                ).astype(np.float32)
    freqs = np.outer(t_ar, inv_freq).astype(np.float32)
    cos = np.cos(freqs).astype(np.float32)
    sin = np.sin(freqs).astype(np.float32)

    # multiplicative 0/1 causal mask (applied to probs AFTER exp)
    masks = np.empty((128, 2, CH), dtype=bf16)
    kk = np.arange(128)[:, None]
    qq = np.arange(CH)[None, :]
    for m in range(2):
        masks[:, m, :] = np.where(m * 128 + kk <= qq, 1.0, 0.0)
    
    ident = np.eye(128, dtype=bf16)

    lam1 = np.exp(np.sum(np.asarray(lambda_q1, np.float32)
                         * np.asarray(lambda_k1, np.float32), axis=-1))
    lam2 = np.exp(np.sum(np.asarray(lambda_q2, np.float32)
                         * np.asarray(lambda_k2, np.float32), axis=-1))
    lam = (lam1 - lam2 + LAMBDA_INIT).astype(np.float32)   # [8]

    in_maps = []
    for c in range(N_CORES):
        base = 2 * c * 384
        w_cols = [
            W_qkv[:, base:base + 128],            # q1
            W_qkv[:, base + 384:base + 512],      # q2
            W_qkv[:, base + 128:base + 256],      # k1
            W_qkv[:, base + 512:base + 640],      # k2
            W_qkv[:, base + 256:base + 384],      # v1
            W_qkv[:, base + 640:base + 768],      # v2
            W_g[:, c * 256:(c + 1) * 256],        # gate
        ]
        wc = np.concatenate(w_cols, axis=1) * WSCALE        # [D, 1024] f32
        w_hi = wc.astype(f8)
        w_lo = (wc - w_hi.astype(np.float32)).astype(f8)
        # pack [D, 2, 1024]: slot0 = LO, slot1 = HI (correction AP order)
        wcat = np.empty((D, 2, 1024), dtype=f8)
        wcat[:, 0, :] = w_lo
        wcat[:, 1, :] = w_hi
        ones = np.zeros((128, 4), dtype=bf16)
        ones[:, 0] = 1.0
        ones[:, 1] = -1.0 / lam[c]
        in_maps.append({
            "xt": xt_p[:, :, :, :, :], "wcat": wcat, "cos": cos, "sin": sin,
            "masks": masks, "ident": ident, "ones": ones,
        })

    return in_maps


def kernel(hidden_states, W_qkv, lambda_q1, lambda_k1, lambda_q2, lambda_k2,
           W_g, **run_kwargs):
    global _NC
    if _NC is None:
        _NC = build()
    in_maps = prep_in_maps(hidden_states, W_qkv, lambda_q1, lambda_k1,
                           lambda_q2, lambda_k2, W_g)
    res = run_bass_kernel_spmd(_NC, in_maps, core_ids=list(range(N_CORES)),
                               **run_kwargs)
    out = np.empty((B, T, D), dtype=np.float32)
    for c in range(N_CORES):
        out[:, :, c * 256:(c + 1) * 256] = res.results[c]["y"]
    if run_kwargs:
        return out, res
    return out

